# revision 39
# baseline (speedup 1.0000x reference)
"""MoE experts kernel (grouped GEMM + SwiGLU) on 8 Trainium2 NeuronCores.

Problem: N=4096 tokens sorted by expert, E=8 experts, H=1024, I=2048, bf16.
  up    = gmm(hiddens, w13)            # [N, 2I]
  gated = silu(up[:, :I]) * up[:, I:]  # [N, I]
  down  = gmm(gated, w2)               # [N, H]

Sharding: expert parallelism. Core e owns expert e's weights and its
contiguous block of tokens (batch_sizes[e] each; 512 in the target
regime). No collectives; tokens are scattered/gathered on the host.

Per-core dataflow (lhsT = stationary operand of nc.tensor.matmul):
  xT    [H, T] shipped PRE-TRANSPOSED from the host (no PE transpose)
  upT   = matmul(lhsT=w13[:, chunk], rhs=xT)  -> PSUM [128, T]  (k = H)
  gatedT[c] = silu(upT_gate) * upT_up         -> SBUF bf16 chunks
  down  = matmul(lhsT=gatedT[c], rhs=w2[c])   -> PSUM [128, 512] (k = I)

Scheduling notes (walrus build: any instruction may carry at most ONE
embedded sync wait; the HWDGE queues share one 8-semaphore pool and
SWDGE has its own 8 — a 9th DMA on a pool carries a semaphore-reuse
wait, so it must be a pure load):
- ALL inputs are concatenated on the host into ONE flat partition-major
  "wall" parameter laid out in consumption order, so every load DMA
  moves one fat contiguous element per partition (each DGE engine
  serves its 8 partitions serially at ~27 GB/s for elements >=3KB —
  thin strided descriptors run several times slower).
- The head region interleaves xT and slab0 per k-subtile, and mm1
  interleaves the pg/pu accumulations per k, so each arriving DMA
  piece immediately feeds 2 matmuls per k-slice. 8 load DMAs ride the
  sync HWDGE ring, strict FIFO (one stream = full rate; splitting
  across both HWDGE rings halves each ring's rate). Piece boundaries
  balance T(piece-complete) + PE-work-remaining about equally, so the
  matmul stream starts as early as the (ramp-limited, ~16-engine-
  straggler-limited) delivery allows and then never stalls.
- No observers: the first LDWEIGHTS touching a slab carries that DMA's
  wait and the start-matmul carries the PSUM-WAR (or xT) wait — one
  embedded wait per instruction.
- WARMUP junk FD=512 matmuls on a memset tile (gpsimd memset, no load
  dependency) bridge the NEFF preamble to piece-1 arrival with no
  PE-idle gap over ~0.3 us: the HAM clock-unthrottle (1.2->2.4 GHz)
  needs a ~3.4 us activity window at >=~85% busy, and a missed window
  costs ~2 us of cold real matmuls.
- Output stores all ride SWDGE: two bulk stores mid-run (mc0-1, then
  mc2 + mc3's first half), then the final PSUM group split [384|128]
  so the 384-cast+store overlap the last 16 matmuls. The split PSUM
  tiles are each a FULL bank: packed 1536B+512B in one bank, the
  bank-aware tracker would serialize the 384-cast (DVE-R) behind the
  128-group (PE-W same bank = fatal collision).
- The tail drain chain is split one-wait-per-drain and dealt across
  the engines, latest-completing sems last.
"""

import sys

if "/opt/trn_rl_repo" not in sys.path:
    sys.path.insert(0, "/opt/trn_rl_repo")

import numpy as np
import ml_dtypes

E = 8
H = 1024
I = 2048
N = 4096
T = N // E          # tokens per expert / core
P = 128
KH = H // P         # 8  k-subtiles for mm1
NI = I // P         # 16 k-subtiles for mm2 / gated chunks
FD = 512            # matmul moving free dim (1 PSUM bank of f32)
# w13 column-slab widths per half: small first so the pipeline starts
# on minimal data while the FIFO streams the rest.
SLABS = (128, 128, 256, 512, 512, 512)
WARMUP = 38         # junk FD=128 matmuls bridging preamble -> first data
# The junk matmuls (107ns each cold) must reach the first real matmul
# (piece-1 complete, ~12.1 us) with no PE-idle gap over ~0.3 us: the
# HAM clock-unthrottle needs one ~3.4 us activity window at >=~85%
# busy, and a miss costs ~2 us of cold (1.2 GHz) real matmuls.
BF16 = ml_dtypes.bfloat16

# Flat per-partition column offsets inside the "wall" parameter (bf16).
# Layout: [ (xk | s0k-gate | s0k-up) for k=0..7 | slab1..slab5 | w2 ] —
# xT and slab0 k-slices interleaved so each head DMA piece lands exactly
# the operands of the next few matmuls (chunk 0 streams pg/pu per-k).
KBLK = T + 2 * P                     # 768 cols per interleaved k-slice
XT_REGION = KH * KBLK                # 6144
_SLAB_BASE = [None]                  # slab0 is interleaved; no flat base
_b = XT_REGION
for _w in SLABS[1:]:
    _SLAB_BASE.append(_b)
    _b += 2 * KH * _w
W2_OFF = _b
W2_LEN = NI * H                      # 16384
WALL_LEN = W2_OFF + W2_LEN           # 53248

# Load-piece boundaries (wall-column coordinates). Each piece is its
# OWN DRAM parameter, stored contiguously: the DMA then reads DRAM at
# a per-partition stride of the piece width (4.6-32KB) instead of the
# 106KB whole-wall row stride — sequential DRAM locality for the HBM
# controller exactly where the head is ramp-bound.
PIECES = [(0, 2 * KBLK),                   # xk0-1 + s0k0-1
          (2 * KBLK, 5 * KBLK),            # xk2-4 + s0k2-4
          (5 * KBLK, _SLAB_BASE[1] + 512),  # xk5-7+s0k5-7+slab1 k0-1
          (_SLAB_BASE[1] + 512, _SLAB_BASE[2] + 1024),  # s1k2-7+s2k0-1
          (_SLAB_BASE[2] + 1024, _SLAB_BASE[3]),        # slab2 k2-7
          (_SLAB_BASE[3], _SLAB_BASE[4]),  # slab 3
          (_SLAB_BASE[4], W2_OFF),         # slabs 4-5
          (W2_OFF, WALL_LEN)]              # w2

_NC_CACHE = {}


def _slab_of(c):
    """Map gated-chunk index c (0..15) -> (slab_idx, col offset in slab)."""
    for si, w in enumerate(SLABS):
        n = w // P
        if c < n:
            return si, c * P
        c -= n
    raise IndexError(c)


def _build_nc(act="silu"):
    import concourse.bass as bass
    import concourse.tile as tile
    from concourse import mybir
    from concourse.vector_clock import ScopedClock, VectorClock
    import bass_rust

    PROC_NAMES = list(bass_rust.PROC_NAMES)

    class SplitDrainTileContext(tile.TileContext):
        """Tail drain emitted as a chain of single-wait drains (the
        compiler rejects instructions with >1 embedded sync wait),
        dealt across engines with late-completing sems last."""

        def _drain_and_barrier(self, tick_clock, wait_clock):
            nc = self.nc
            gclock = tick_clock.global_clock
            n = len(gclock)

            def prio(p):
                name = PROC_NAMES[p] if p < len(PROC_NAMES) else ""
                if name.startswith("DMAHW"):
                    return (0, p)          # loads + the final store
                if name.startswith("DMASW"):
                    return (2, p)          # bulk stores
                if name == "Pool":
                    return (3, p)          # gates on last SW store issue
                return (1, p)              # engine ticks / sequencers

            procs = sorted((p for p in range(n) if gclock[p] > 0), key=prio)
            lanes = [nc.sync, nc.tensor, nc.vector, nc.scalar]
            for i, p in enumerate(procs):
                masked = VectorClock([gclock[q] if q == p else 0
                                      for q in range(n)])
                d = lanes[i % len(lanes)].drain()
                wait_clock.add_sem_waits(d.ins, ScopedClock({None: masked}))
            nc.all_engine_barrier()
            assert self.sems is not None
            popped = nc._tile_sem_poison_stack.pop()
            assert popped is self._sem_poison
            nc.clear_and_free_semaphores(list(self.sems.allocated().values()))

    nc = bass.Bass()
    bf = mybir.dt.bfloat16
    f32 = mybir.dt.float32

    wps = [nc.declare_dram_parameter(f"wp{i}", [P, b - a], bf,
                                     isOutput=False)
           for i, (a, b) in enumerate(PIECES)]
    # out[p, a*H + h] = down[a*P + p, h]  (host untangles)
    out = nc.declare_dram_parameter("out", [P, (T // P) * H], bf,
                                    isOutput=True)

    fn = (mybir.ActivationFunctionType.Silu if act == "silu"
          else mybir.ActivationFunctionType.Sigmoid)

    with SplitDrainTileContext(nc) as tc:
        with (
            tc.tile_pool(name="persist", bufs=1) as persist,
            tc.tile_pool(name="sgp", bufs=16) as sgp,
            tc.tile_pool(name="gtp", bufs=16) as gtp,
            tc.tile_pool(name="tch", bufs=16) as tch,
            tc.tile_pool(name="otp", bufs=1) as otp,
            tc.tile_pool(name="pst", bufs=1, space="PSUM") as pst,
            tc.tile_pool(name="ps1", bufs=2, space="PSUM") as ps1,
            tc.tile_pool(name="ps2", bufs=2, space="PSUM") as ps2,
        ):
            # ---- Load plan: single sync HWDGE queue, consumption order,
            # every DMA one fat contiguous element per partition.
            # Minimal [P,128] memset on gpsimd (~150ns) so the junk
            # warm-up matmuls start right at the preamble end, pulling
            # the HAM clock-flip as early as possible.
            warm = persist.tile([P, P], bf, tag="warm")
            nc.gpsimd.memset(warm[:], 0.25)

            ws = persist.tile([P, WALL_LEN], bf, tag="wall")
            # 8 loads on the sync HWDGE ring (full 8-sem pool; all
            # stores ride SWDGE), strict FIFO, consumption order.
            # Piece boundaries balance T(piece) + PE-work-remaining so
            # every head piece binds the matmul-stream end equally
            # (~94.5 us): the slowest of the 16 DGE engines lags ~1-2 us
            # behind the mean, and a consumer waits all 16 (sem>=16).
            # 8 loads on the sync HWDGE ring (full 8-sem HW pool; all
            # stores ride SWDGE), strict FIFO, consumption order.
            # Piece boundaries balance T(piece) + PE-work-remaining so
            # every head piece binds the matmul-stream end about
            # equally; the slowest of the 16 DGE engines lags ~1-2 us
            # behind the mean and a consumer waits all 16 (sem>=16).
            # (Splitting loads across both HWDGE rings halves each
            # ring's rate — the engines round-robin between rings at
            # packet granularity — and measures far worse.)
            for wp, (a, b) in zip(wps, PIECES):
                nc.sync.dma_start(ws[:, a:b], wp[:, :])

            def xk(k):
                return ws[:, k * KBLK:k * KBLK + T]

            def gsl(si, k, co):
                if si == 0:
                    base = k * KBLK + T
                else:
                    base = _SLAB_BASE[si] + k * 2 * SLABS[si]
                return ws[:, base + co:base + co + P]

            def usl(si, k, co):
                if si == 0:
                    base = k * KBLK + T + P
                else:
                    base = _SLAB_BASE[si] + k * 2 * SLABS[si] + SLABS[si]
                return ws[:, base + co:base + co + P]

            def w2sl(kc, col, wd):
                base = W2_OFF + kc * H + col
                return ws[:, base:base + wd]

            # ---- PE warm-up (junk FD=128 matmuls) ----
            # Real matmul-mode work counts as PE-busy for the HAM, so
            # these start the 3.4 us activity window immediately after
            # the NEFF preamble while the first DMA piece is in flight;
            # the 1.2->2.4 GHz flip lands at about the same time as the
            # first real matmuls. Output goes to a scratch PSUM bank.
            dummy = pst.tile([P, FD], f32, tag="dummy")
            for _ in range(WARMUP):
                nc.tensor.matmul(dummy[:, 0:P], warm[:], warm[:],
                                 start=True, stop=True)

            # No observers needed: the first LDWEIGHTS touching a slab
            # carries that DMA's wait and the start-matmul carries the
            # PSUM WAR (or xT) wait — one embedded wait per instruction.

            # ---- mm1 + SwiGLU over 16 gate/up column-chunk pairs ----
            # pg/pu accumulation interleaved per k so chunk 0 consumes
            # the head DMA pieces in arrival order (2 matmuls per
            # k-slice as it lands).
            gts = []
            for c in range(NI):
                si, co = _slab_of(c)
                pg = ps1.tile([P, T], f32, tag="pg")
                pu = ps1.tile([P, T], f32, tag="pu")
                for k in range(KH):
                    nc.tensor.matmul(
                        pg[:], gsl(si, k, co), xk(k),
                        start=(k == 0), stop=(k == KH - 1),
                    )
                    nc.tensor.matmul(
                        pu[:], usl(si, k, co), xk(k),
                        start=(k == 0), stop=(k == KH - 1),
                    )
                sg = sgp.tile([P, T], bf, tag="sg")
                nc.scalar.activation(sg[:], pg[:], fn)
                # A DVE instruction may carry one sync wait: this tiny copy
                # takes the ACT wait so the gating mul below only needs PE.
                touch = tch.tile([P, 1], bf, tag="touch")
                nc.vector.tensor_copy(touch[:], sg[:, 0:1])
                gt = gtp.tile([P, T], bf, tag="gt")
                nc.vector.scalar_tensor_tensor(
                    gt[:], pu[:], 1.0, sg[:],
                    mybir.AluOpType.mult, mybir.AluOpType.mult,
                )
                gts.append(gt)

            # ---- mm2: down[mc*P:, :] = gatedT.T @ w2 ----
            # obuf / out are flat [P, mc*H + h]. One bulk SWDGE store
            # covers everything before the final split group; the very
            # last PSUM group is split [384|128] so the tail cast+store
            # is only 32KB deep.
            obuf = otp.tile([P, (T // P) * H], bf, tag="obuf")
            for mc in range(T // P):  # 4
                for nh in range(H // FD):  # 2
                    final = (mc == T // P - 1) and (nh == H // FD - 1)
                    widths = [FD] if not final else [3 * FD // 4, FD // 4]
                    col0 = nh * FD
                    for j, wd in enumerate(widths):
                        # always allocate a full PSUM bank: a 1536B+512B
                        # pair packed into one bank would serialize the
                        # first cast behind the second group's matmuls
                        # (DVE-R + PE-W of one bank is a fatal collision,
                        # so the tracker orders them)
                        pdt = ps2.tile([P, FD], f32, tag="pd")
                        pdv = pdt[:, 0:wd]
                        for kc in range(NI):
                            nc.tensor.matmul(
                                pdv,
                                gts[kc][:, mc * P:(mc + 1) * P],
                                w2sl(kc, col0, wd),
                                start=(kc == 0), stop=(kc == NI - 1),
                            )
                        ob = obuf[:, mc * H + col0:mc * H + col0 + wd]
                        nc.vector.tensor_copy(ob, pdv)
                        if final:
                            # one store per half right after its cast
                            # (SWDGE; the HW ring's 8 sems are all loads)
                            nc.gpsimd.dma_start(
                                out[:, mc * H + col0:mc * H + col0 + wd],
                                ob)
                        col0 += wd
                    # One bulk store (SWDGE) for everything before the
                    # final split group: fewer issue slots serialized on
                    # the gpsimd queue ahead of the critical last store.
                    if mc == 3 and nh == 0:
                        nc.gpsimd.dma_start(out[:, 0:3 * H + FD],
                                            obuf[:, 0:3 * H + FD])

    return nc


def _get_nc():
    if "nc" not in _NC_CACHE:
        _NC_CACHE["nc"] = _build_nc()
    return _NC_CACHE["nc"]


def _prep_w2(w2_e):
    """w2_e [I, H] -> [P, NI*H] partition-major."""
    return w2_e.reshape(NI, P, H).transpose(1, 0, 2).reshape(P, NI * H)


def _make_in_map(tokens, w13_e, w2_e):
    """Per-core input dict in the kernel's current DRAM layout."""
    xT = np.ascontiguousarray(np.asarray(tokens).astype(BF16).T)  # [H, T]
    xtp = xT.reshape(KH, P, T)                   # [KH, P, T] per k-slice
    w4 = np.asarray(w13_e).astype(BF16).reshape(KH, P, 2 * I)
    # interleaved head: per k: [xk | slab0 gate | slab0 up]
    head = np.concatenate(
        [xtp, w4[:, :, 0:P], w4[:, :, I:I + P]], axis=2)  # [KH, P, KBLK]
    parts = [head.transpose(1, 0, 2).reshape(P, XT_REGION)]
    off = P
    for wdt in SLABS[1:]:
        g = w4[:, :, off:off + wdt]              # [KH, P, wdt]
        u = w4[:, :, I + off:I + off + wdt]
        gu = np.concatenate([g, u], axis=2)      # [KH, P, 2*wdt]
        parts.append(gu.transpose(1, 0, 2).reshape(P, -1))
        off += wdt
    parts.append(_prep_w2(np.asarray(w2_e).astype(BF16)))
    wall = np.concatenate(parts, axis=1)
    return {f"wp{i}": np.ascontiguousarray(wall[:, a:b])
            for i, (a, b) in enumerate(PIECES)}


def kernel(bs, hiddens, w13_weight, w2_weight, batch_sizes, **_ignored):
    from concourse.bass_utils import run_bass_kernel_spmd

    hiddens = np.asarray(hiddens)
    w13_weight = np.asarray(w13_weight)
    w2_weight = np.asarray(w2_weight)
    batch_sizes = np.asarray(batch_sizes).astype(np.int64)

    in_dtype = hiddens.dtype
    x = np.ascontiguousarray(hiddens.astype(BF16))
    w13 = np.ascontiguousarray(w13_weight.astype(BF16))
    w2 = np.ascontiguousarray(w2_weight.astype(BF16))

    assert batch_sizes.shape == (E,) and int(batch_sizes.sum()) == N, (
        "kernel compiled for 8 experts x 4096 tokens"
    )

    offsets = np.concatenate([[0], np.cumsum(batch_sizes)])
    uniform = bool((batch_sizes == T).all())

    in_maps = []
    for e in range(E):
        if uniform:
            tok = x[e * T:(e + 1) * T]
        else:
            blk = x[offsets[e]:offsets[e + 1]]
            assert blk.shape[0] <= T, "per-expert batch exceeds compiled T"
            tok = np.zeros((T, H), dtype=BF16)
            tok[: blk.shape[0]] = blk
        in_maps.append(_make_in_map(tok, w13[e], w2[e]))

    nc = _get_nc()
    results = run_bass_kernel_spmd(nc, in_maps, list(range(E))).results

    out_full = np.empty((N, H), dtype=BF16)
    for e in range(E):
        oe = np.asarray(results[e]["out"])           # [P, (T//P)*H]
        blk = oe.reshape(P, T // P, H).transpose(1, 0, 2).reshape(T, H)
        if uniform:
            out_full[e * T:(e + 1) * T] = blk
        else:
            nb = int(batch_sizes[e])
            out_full[offsets[e]:offsets[e + 1]] = blk[:nb]

    return out_full.astype(in_dtype)



# revision 41
# speedup vs baseline: 1.0086x; 1.0086x over previous
"""MoE experts kernel (grouped GEMM + SwiGLU) on 8 Trainium2 NeuronCores.

Problem: N=4096 tokens sorted by expert, E=8 experts, H=1024, I=2048, bf16.
  up    = gmm(hiddens, w13)            # [N, 2I]
  gated = silu(up[:, :I]) * up[:, I:]  # [N, I]
  down  = gmm(gated, w2)               # [N, H]

Sharding: expert parallelism. Core e owns expert e's weights and its
contiguous block of tokens (batch_sizes[e] each; 512 in the target
regime). No collectives; tokens are scattered/gathered on the host.

Per-core dataflow (lhsT = stationary operand of nc.tensor.matmul):
  xT    [H, T] shipped PRE-TRANSPOSED from the host (no PE transpose)
  upT   = matmul(lhsT=w13[:, chunk], rhs=xT)  -> PSUM [128, T]  (k = H)
  gatedT[c] = silu(upT_gate) * upT_up         -> SBUF bf16 chunks
  down  = matmul(lhsT=gatedT[c], rhs=w2[c])   -> PSUM [128, 512] (k = I)

Scheduling notes (walrus build: any instruction may carry at most ONE
embedded sync wait; the HWDGE queues share one 8-semaphore pool and
SWDGE has its own 8 — a 9th DMA on a pool carries a semaphore-reuse
wait, so it must be a pure load):
- ALL inputs are concatenated on the host into ONE flat partition-major
  "wall" parameter laid out in consumption order, so every load DMA
  moves one fat contiguous element per partition (each DGE engine
  serves its 8 partitions serially at ~27 GB/s for elements >=3KB —
  thin strided descriptors run several times slower).
- The head region interleaves xT and slab0 per k-subtile, and mm1
  interleaves the pg/pu accumulations per k, so each arriving DMA
  piece immediately feeds 2 matmuls per k-slice. 8 load DMAs ride the
  sync HWDGE ring, strict FIFO (one stream = full rate; splitting
  across both HWDGE rings halves each ring's rate). Piece boundaries
  balance T(piece-complete) + PE-work-remaining about equally, so the
  matmul stream starts as early as the (ramp-limited, ~16-engine-
  straggler-limited) delivery allows and then never stalls.
- No observers: the first LDWEIGHTS touching a slab carries that DMA's
  wait and the start-matmul carries the PSUM-WAR (or xT) wait — one
  embedded wait per instruction.
- WARMUP junk FD=512 matmuls on a memset tile (gpsimd memset, no load
  dependency) bridge the NEFF preamble to piece-1 arrival with no
  PE-idle gap over ~0.3 us: the HAM clock-unthrottle (1.2->2.4 GHz)
  needs a ~3.4 us activity window at >=~85% busy, and a missed window
  costs ~2 us of cold real matmuls.
- Output stores all ride SWDGE: two bulk stores mid-run (mc0-1, then
  mc2 + mc3's first half), then the final PSUM group split [384|128]
  so the 384-cast+store overlap the last 16 matmuls. The split PSUM
  tiles are each a FULL bank: packed 1536B+512B in one bank, the
  bank-aware tracker would serialize the 384-cast (DVE-R) behind the
  128-group (PE-W same bank = fatal collision).
- The tail drain chain is split one-wait-per-drain and dealt across
  the engines, latest-completing sems last.
"""

import sys

if "/opt/trn_rl_repo" not in sys.path:
    sys.path.insert(0, "/opt/trn_rl_repo")

import numpy as np
import ml_dtypes

E = 8
H = 1024
I = 2048
N = 4096
T = N // E          # tokens per expert / core
P = 128
KH = H // P         # 8  k-subtiles for mm1
NI = I // P         # 16 k-subtiles for mm2 / gated chunks
FD = 512            # matmul moving free dim (1 PSUM bank of f32)
# w13 column-slab widths per half: small first so the pipeline starts
# on minimal data while the FIFO streams the rest.
SLABS = (128, 128, 256, 512, 512, 512)
WARMUP = 42         # junk FD=128 matmuls bridging preamble -> first data
# The junk matmuls (107ns each cold) must reach the first real matmul
# (piece-1 complete, ~12.1 us) with no PE-idle gap over ~0.3 us: the
# HAM clock-unthrottle needs one ~3.4 us activity window at >=~85%
# busy, and a miss costs ~2 us of cold (1.2 GHz) real matmuls.
BF16 = ml_dtypes.bfloat16

# Flat per-partition column offsets inside the "wall" parameter (bf16).
# Layout: [ (xk | s0k-gate | s0k-up) for k=0..7 | slab1..slab5 | w2 ] —
# xT and slab0 k-slices interleaved so each head DMA piece lands exactly
# the operands of the next few matmuls (chunk 0 streams pg/pu per-k).
KBLK = T + 2 * P                     # 768 cols per interleaved k-slice
XT_REGION = KH * KBLK                # 6144
_SLAB_BASE = [None]                  # slab0 is interleaved; no flat base
_b = XT_REGION
for _w in SLABS[1:]:
    _SLAB_BASE.append(_b)
    _b += 2 * KH * _w
W2_OFF = _b
W2_LEN = NI * H                      # 16384
WALL_LEN = W2_OFF + W2_LEN           # 53248

# Load-piece boundaries (wall-column coordinates). Each piece is its
# OWN DRAM parameter, stored contiguously: the DMA then reads DRAM at
# a per-partition stride of the piece width (4.6-32KB) instead of the
# 106KB whole-wall row stride — sequential DRAM locality for the HBM
# controller exactly where the head is ramp-bound.
PIECES = [(0, 3 * KBLK),                   # xk0-2 + s0k0-2
          (3 * KBLK, 6 * KBLK),            # xk3-5 + s0k3-5
          (6 * KBLK, _SLAB_BASE[1] + 512),  # xk6-7+s0k6-7+slab1 k0-1
          (_SLAB_BASE[1] + 512, _SLAB_BASE[2] + 1024),  # s1k2-7+s2k0-1
          (_SLAB_BASE[2] + 1024, _SLAB_BASE[3]),        # slab2 k2-7
          (_SLAB_BASE[3], _SLAB_BASE[4]),  # slab 3
          (_SLAB_BASE[4], W2_OFF),         # slabs 4-5
          (W2_OFF, WALL_LEN)]              # w2

_NC_CACHE = {}


def _slab_of(c):
    """Map gated-chunk index c (0..15) -> (slab_idx, col offset in slab)."""
    for si, w in enumerate(SLABS):
        n = w // P
        if c < n:
            return si, c * P
        c -= n
    raise IndexError(c)


def _build_nc(act="silu"):
    import concourse.bass as bass
    import concourse.tile as tile
    from concourse import mybir
    from concourse.vector_clock import ScopedClock, VectorClock
    import bass_rust

    PROC_NAMES = list(bass_rust.PROC_NAMES)

    class SplitDrainTileContext(tile.TileContext):
        """Tail drain emitted as a chain of single-wait drains (the
        compiler rejects instructions with >1 embedded sync wait),
        dealt across engines with late-completing sems last."""

        def _drain_and_barrier(self, tick_clock, wait_clock):
            nc = self.nc
            gclock = tick_clock.global_clock
            n = len(gclock)

            def prio(p):
                name = PROC_NAMES[p] if p < len(PROC_NAMES) else ""
                if name.startswith("DMAHW"):
                    return (0, p)          # loads + the final store
                if name.startswith("DMASW"):
                    return (2, p)          # bulk stores
                if name == "Pool":
                    return (3, p)          # gates on last SW store issue
                return (1, p)              # engine ticks / sequencers

            procs = sorted((p for p in range(n) if gclock[p] > 0), key=prio)
            lanes = [nc.sync, nc.tensor, nc.vector, nc.scalar]
            for i, p in enumerate(procs):
                masked = VectorClock([gclock[q] if q == p else 0
                                      for q in range(n)])
                d = lanes[i % len(lanes)].drain()
                wait_clock.add_sem_waits(d.ins, ScopedClock({None: masked}))
            nc.all_engine_barrier()
            assert self.sems is not None
            popped = nc._tile_sem_poison_stack.pop()
            assert popped is self._sem_poison
            nc.clear_and_free_semaphores(list(self.sems.allocated().values()))

    nc = bass.Bass()
    bf = mybir.dt.bfloat16
    f32 = mybir.dt.float32

    wps = [nc.declare_dram_parameter(f"wp{i}", [P, b - a], bf,
                                     isOutput=False)
           for i, (a, b) in enumerate(PIECES)]
    # out[p, a*H + h] = down[a*P + p, h]  (host untangles)
    out = nc.declare_dram_parameter("out", [P, (T // P) * H], bf,
                                    isOutput=True)

    fn = (mybir.ActivationFunctionType.Silu if act == "silu"
          else mybir.ActivationFunctionType.Sigmoid)

    with SplitDrainTileContext(nc) as tc:
        with (
            tc.tile_pool(name="persist", bufs=1) as persist,
            tc.tile_pool(name="sgp", bufs=16) as sgp,
            tc.tile_pool(name="gtp", bufs=16) as gtp,
            tc.tile_pool(name="tch", bufs=16) as tch,
            tc.tile_pool(name="otp", bufs=1) as otp,
            tc.tile_pool(name="pst", bufs=1, space="PSUM") as pst,
            tc.tile_pool(name="ps1", bufs=2, space="PSUM") as ps1,
            tc.tile_pool(name="ps2", bufs=2, space="PSUM") as ps2,
        ):
            # ---- Load plan: single sync HWDGE queue, consumption order,
            # every DMA one fat contiguous element per partition.
            # Minimal [P,128] memset on gpsimd (~150ns) so the junk
            # warm-up matmuls start right at the preamble end, pulling
            # the HAM clock-flip as early as possible.
            warm = persist.tile([P, P], bf, tag="warm")
            nc.gpsimd.memset(warm[:], 0.25)

            ws = persist.tile([P, WALL_LEN], bf, tag="wall")
            # 8 loads on the sync HWDGE ring (full 8-sem pool; all
            # stores ride SWDGE), strict FIFO, consumption order.
            # Piece boundaries balance T(piece) + PE-work-remaining so
            # every head piece binds the matmul-stream end equally
            # (~94.5 us): the slowest of the 16 DGE engines lags ~1-2 us
            # behind the mean, and a consumer waits all 16 (sem>=16).
            # 8 loads on the sync HWDGE ring (full 8-sem HW pool; all
            # stores ride SWDGE), strict FIFO, consumption order.
            # Piece boundaries balance T(piece) + PE-work-remaining so
            # every head piece binds the matmul-stream end about
            # equally; the slowest of the 16 DGE engines lags ~1-2 us
            # behind the mean and a consumer waits all 16 (sem>=16).
            # (Splitting loads across both HWDGE rings halves each
            # ring's rate — the engines round-robin between rings at
            # packet granularity — and measures far worse.)
            for wp, (a, b) in zip(wps, PIECES):
                nc.sync.dma_start(ws[:, a:b], wp[:, :])

            def xk(k):
                return ws[:, k * KBLK:k * KBLK + T]

            def gsl(si, k, co):
                if si == 0:
                    base = k * KBLK + T
                else:
                    base = _SLAB_BASE[si] + k * 2 * SLABS[si]
                return ws[:, base + co:base + co + P]

            def usl(si, k, co):
                if si == 0:
                    base = k * KBLK + T + P
                else:
                    base = _SLAB_BASE[si] + k * 2 * SLABS[si] + SLABS[si]
                return ws[:, base + co:base + co + P]

            def w2sl(kc, col, wd):
                base = W2_OFF + kc * H + col
                return ws[:, base:base + wd]

            # ---- PE warm-up (junk FD=128 matmuls) ----
            # Real matmul-mode work counts as PE-busy for the HAM, so
            # these start the 3.4 us activity window immediately after
            # the NEFF preamble while the first DMA piece is in flight;
            # the 1.2->2.4 GHz flip lands at about the same time as the
            # first real matmuls. Output goes to a scratch PSUM bank.
            dummy = pst.tile([P, FD], f32, tag="dummy")
            for _ in range(WARMUP):
                nc.tensor.matmul(dummy[:, 0:P], warm[:], warm[:],
                                 start=True, stop=True)

            # No observers needed: the first LDWEIGHTS touching a slab
            # carries that DMA's wait and the start-matmul carries the
            # PSUM WAR (or xT) wait — one embedded wait per instruction.

            # ---- mm1 + SwiGLU over 16 gate/up column-chunk pairs ----
            # pg/pu accumulation interleaved per k so chunk 0 consumes
            # the head DMA pieces in arrival order (2 matmuls per
            # k-slice as it lands).
            gts = []
            for c in range(NI):
                si, co = _slab_of(c)
                pg = ps1.tile([P, T], f32, tag="pg")
                pu = ps1.tile([P, T], f32, tag="pu")
                for k in range(KH):
                    nc.tensor.matmul(
                        pg[:], gsl(si, k, co), xk(k),
                        start=(k == 0), stop=(k == KH - 1),
                    )
                    nc.tensor.matmul(
                        pu[:], usl(si, k, co), xk(k),
                        start=(k == 0), stop=(k == KH - 1),
                    )
                sg = sgp.tile([P, T], bf, tag="sg")
                nc.scalar.activation(sg[:], pg[:], fn)
                # A DVE instruction may carry one sync wait: this tiny copy
                # takes the ACT wait so the gating mul below only needs PE.
                touch = tch.tile([P, 1], bf, tag="touch")
                nc.vector.tensor_copy(touch[:], sg[:, 0:1])
                gt = gtp.tile([P, T], bf, tag="gt")
                nc.vector.scalar_tensor_tensor(
                    gt[:], pu[:], 1.0, sg[:],
                    mybir.AluOpType.mult, mybir.AluOpType.mult,
                )
                gts.append(gt)

            # ---- mm2: down[mc*P:, :] = gatedT.T @ w2 ----
            # obuf / out are flat [P, mc*H + h]. One bulk SWDGE store
            # covers everything before the final split group; the very
            # last PSUM group is split [384|128] so the tail cast+store
            # is only 32KB deep.
            obuf = otp.tile([P, (T // P) * H], bf, tag="obuf")
            for mc in range(T // P):  # 4
                for nh in range(H // FD):  # 2
                    final = (mc == T // P - 1) and (nh == H // FD - 1)
                    widths = [FD] if not final else [3 * FD // 4, FD // 4]
                    col0 = nh * FD
                    for j, wd in enumerate(widths):
                        # always allocate a full PSUM bank: a 1536B+512B
                        # pair packed into one bank would serialize the
                        # first cast behind the second group's matmuls
                        # (DVE-R + PE-W of one bank is a fatal collision,
                        # so the tracker orders them)
                        pdt = ps2.tile([P, FD], f32, tag="pd")
                        pdv = pdt[:, 0:wd]
                        for kc in range(NI):
                            nc.tensor.matmul(
                                pdv,
                                gts[kc][:, mc * P:(mc + 1) * P],
                                w2sl(kc, col0, wd),
                                start=(kc == 0), stop=(kc == NI - 1),
                            )
                        ob = obuf[:, mc * H + col0:mc * H + col0 + wd]
                        nc.vector.tensor_copy(ob, pdv)
                        if final:
                            # one store per half right after its cast
                            # (SWDGE; the HW ring's 8 sems are all loads)
                            nc.gpsimd.dma_start(
                                out[:, mc * H + col0:mc * H + col0 + wd],
                                ob)
                        col0 += wd
                    # One bulk store (SWDGE) for everything before the
                    # final split group: fewer issue slots serialized on
                    # the gpsimd queue ahead of the critical last store.
                    if mc == 3 and nh == 0:
                        nc.gpsimd.dma_start(out[:, 0:3 * H + FD],
                                            obuf[:, 0:3 * H + FD])

    return nc


def _get_nc():
    if "nc" not in _NC_CACHE:
        _NC_CACHE["nc"] = _build_nc()
    return _NC_CACHE["nc"]


def _prep_w2(w2_e):
    """w2_e [I, H] -> [P, NI*H] partition-major."""
    return w2_e.reshape(NI, P, H).transpose(1, 0, 2).reshape(P, NI * H)


def _make_in_map(tokens, w13_e, w2_e):
    """Per-core input dict in the kernel's current DRAM layout."""
    xT = np.ascontiguousarray(np.asarray(tokens).astype(BF16).T)  # [H, T]
    xtp = xT.reshape(KH, P, T)                   # [KH, P, T] per k-slice
    w4 = np.asarray(w13_e).astype(BF16).reshape(KH, P, 2 * I)
    # interleaved head: per k: [xk | slab0 gate | slab0 up]
    head = np.concatenate(
        [xtp, w4[:, :, 0:P], w4[:, :, I:I + P]], axis=2)  # [KH, P, KBLK]
    parts = [head.transpose(1, 0, 2).reshape(P, XT_REGION)]
    off = P
    for wdt in SLABS[1:]:
        g = w4[:, :, off:off + wdt]              # [KH, P, wdt]
        u = w4[:, :, I + off:I + off + wdt]
        gu = np.concatenate([g, u], axis=2)      # [KH, P, 2*wdt]
        parts.append(gu.transpose(1, 0, 2).reshape(P, -1))
        off += wdt
    parts.append(_prep_w2(np.asarray(w2_e).astype(BF16)))
    wall = np.concatenate(parts, axis=1)
    return {f"wp{i}": np.ascontiguousarray(wall[:, a:b])
            for i, (a, b) in enumerate(PIECES)}


def kernel(bs, hiddens, w13_weight, w2_weight, batch_sizes, **_ignored):
    from concourse.bass_utils import run_bass_kernel_spmd

    hiddens = np.asarray(hiddens)
    w13_weight = np.asarray(w13_weight)
    w2_weight = np.asarray(w2_weight)
    batch_sizes = np.asarray(batch_sizes).astype(np.int64)

    in_dtype = hiddens.dtype
    x = np.ascontiguousarray(hiddens.astype(BF16))
    w13 = np.ascontiguousarray(w13_weight.astype(BF16))
    w2 = np.ascontiguousarray(w2_weight.astype(BF16))

    assert batch_sizes.shape == (E,) and int(batch_sizes.sum()) == N, (
        "kernel compiled for 8 experts x 4096 tokens"
    )

    offsets = np.concatenate([[0], np.cumsum(batch_sizes)])
    uniform = bool((batch_sizes == T).all())

    in_maps = []
    for e in range(E):
        if uniform:
            tok = x[e * T:(e + 1) * T]
        else:
            blk = x[offsets[e]:offsets[e + 1]]
            assert blk.shape[0] <= T, "per-expert batch exceeds compiled T"
            tok = np.zeros((T, H), dtype=BF16)
            tok[: blk.shape[0]] = blk
        in_maps.append(_make_in_map(tok, w13[e], w2[e]))

    nc = _get_nc()
    results = run_bass_kernel_spmd(nc, in_maps, list(range(E))).results

    out_full = np.empty((N, H), dtype=BF16)
    for e in range(E):
        oe = np.asarray(results[e]["out"])           # [P, (T//P)*H]
        blk = oe.reshape(P, T // P, H).transpose(1, 0, 2).reshape(T, H)
        if uniform:
            out_full[e * T:(e + 1) * T] = blk
        else:
            nb = int(batch_sizes[e])
            out_full[offsets[e]:offsets[e + 1]] = blk[:nb]

    return out_full.astype(in_dtype)

